# revision 8
# baseline (speedup 1.0000x reference)
"""AR-GAS-Net Trainium2 kernel v4 (8-core SPMD, data-parallel over batch).

Per core (BC=32768 rows, 256 row-tiles of 128):
  - bf16 MLP on TensorE (as v3): H padded 100->128, zero-bias net,
    x host-cast bf16, CHUNK=1024-row chunks.
  - 64-step GAS scan, 2 lanes of 128 row-tiles with skew S in a
    tick-interleaved layout: tick t's slab is a CONTIGUOUS [128, 256]
    region whose left half is lane0@k=t and right half is lane1@k=t-S.
    All scan ops are packed 2D APs (3D strided APs run ~3.5x slower on
    the DVE: 716ns vs 202ns per 256 cols, measured).
  - chain per tick (all DVE, no cross-engine hops):
      e   = dp - mu                  TT   (2x_1p, ~202ns @256c)
      f   = Q * recip1NR(e^2 + Q)    custom FQR  (1x, ~332ns; 8/8 uop
                                     stages, +-0.17% recip err)
      m1  = e * f                    TT   (~202ns)
      mu' = (A*m1 + o_mu) + b_mu*mu  custom AFF_AFF2
      Q'  = (-Ct*f + Ct+D)*Q + wt    custom QF
  - tails (sg=sqrt(Q'/nu); out = dp*sg + mu') bulk-pipelined: sqrt on
    ACT, mult/add on DVE over packed multi-tick regions.
  - MLP relus on ACT; dp-evac copies on Pool; x DMA on Sync queue.
"""

import os
import numpy as np

import concourse.bass as bass
import concourse.bacc as bacc
import concourse.mybir as mybir
from concourse import tile
from concourse.bass_utils import run_bass_kernel_spmd

f32 = mybir.dt.float32
bf16 = mybir.dt.bfloat16
AF = mybir.ActivationFunctionType
ALU = mybir.AluOpType

B, K, D_IN, H = 262144, 64, 200, 100
HP = 128                    # padded hidden width (zero-bias net)
NCORES = 8
BC = B // NCORES            # 32768 rows per core
P = 128
T = BC // P                 # 256 row-tiles
LW = 128                    # lane width (row-tiles per lane)
S = int(os.environ.get("ARGAS_S", "32"))   # lane skew in ticks
NT = K + S                  # number of ticks
CHUNK = 1024                # MLP chunk rows
NCH = BC // CHUNK           # 32 chunks
CPL = NCH // 2              # chunks per lane (16)
XBUFS = int(os.environ.get("ARGAS_XBUFS", "6"))
MM_N = int(os.environ.get("ARGAS_MMN", "512"))
EVAC = os.environ.get("ARGAS_EVAC", "act")       # act|pool (pool can't
                                                 # read PSUM -> act only)
RELU2 = os.environ.get("ARGAS_RELU2", "dve0")    # act|dve0 (lane0 on DVE)
TAILENG = os.environ.get("ARGAS_TAILENG", "pool")  # pool|dve
TAILB = int(os.environ.get("ARGAS_TAILB", "8"))  # tail bulk ticks

# recip seed constants (optimal for the 1-NR variant too; see dve_ops)
_RC0, _RC1 = -0.23549792, 2.0017324

# ---------------------------------------------------------------- custom ops
_CUSTOM = None


def _register_custom_ops():
    global _CUSTOM
    if _CUSTOM is not None:
        return _CUSTOM
    import concourse.dve_ops as dve_ops
    from concourse.dve_spec import (
        Spec, Src0, Src1, C0, C1, C2, sq, lower, Bin, AluOp)
    from concourse.dve_uop import DveOpSpec

    def _ref_fqr(in0, in1, c0, c1, c2):
        d = (in0.astype(np.float32) ** 2 + in1).astype(np.float32)
        nx = (~d.view(np.int32)).view(np.float32)
        y0 = nx * np.float32(c0)
        y1 = (y0 * (np.float32(c1) - d * y0)).astype(np.float32)
        return in1 * y1

    _d = sq(Src0) + Src1
    _nx = Bin(AluOp.BITWISE_NOT, _d, _d)
    _y0 = _nx * C0
    _y1 = _y0 * (C1 - _d * _y0)

    defs = [
        # f = Q * recip1NR(e*e + Q); Src0=e, Src1=Q
        ("ARGAS_FQR", Spec(
            body=_y1 * Src1,
            reference=_ref_fqr)),
        # mu' = (m1*A + o_mu) + mu*b_mu  (Src0=mu, Src1=m1)
        ("ARGAS_AFF_AFF2", Spec(
            body=(Src1 * C0 + C1) + Src0 * C2,
            reference=lambda in0, in1, c0, c1, c2:
                (in1.astype(np.float32) * c0 + c1) + in0 * c2)),
        # Q' = ((f*C0 + C1))*Q + C2   (Src0=f, Src1=Q)
        ("ARGAS_QF", Spec(
            body=(Src0 * C0 + C1) * Src1 + C2,
            reference=lambda in0, in1, c0, c1, c2:
                (in0.astype(np.float32) * c0 + c1) * in1 + c2)),
    ]
    ops = {}
    for name, spec in defs:
        if name not in dve_ops._SUB_OPCODE_FOR_NAME:
            row = dve_ops._CUSTOM_DVE_ROW_BASE + len(dve_ops.OPS)
            assert row < 0x20, "custom-DVE row overflow"
            dve_ops._SUB_OPCODE_FOR_NAME[name] = row
        tmp = {}
        for ver in ("v3", "v4"):
            try:
                s = DveOpSpec(
                    name=name,
                    opcode=dve_ops.get_dve_sub_opcode(name),
                    uops=lower(spec, ver=ver),
                    rd1_en=True,
                )
                tmp[ver] = s.sha(ver)
            except Exception:
                pass
        op = dve_ops.DveOp(name, spec, subdim=False, uops_sha=tmp)
        if all(o.name != name for o in dve_ops.OPS):
            dve_ops.OPS.append(op)
        dve_ops.CUSTOM_DVE_SPECS[name] = spec
        ops[name] = op
    _CUSTOM = ops
    return _CUSTOM


# ---------------------------------------------------------------- builder
def build_nc(sc):
    cust = _register_custom_ops()
    nc = bacc.Bacc(None)

    xT = nc.dram_tensor("xT", [D_IN, BC], bf16, kind="ExternalInput")
    W1d = nc.dram_tensor("W1e", [D_IN, HP], bf16, kind="ExternalInput")
    W2d = nc.dram_tensor("W2e", [HP, HP], bf16, kind="ExternalInput")
    W3d = nc.dram_tensor("W3e", [HP, K], bf16, kind="ExternalInput")
    # per-lane init state, [P, LW] each
    mu0d = nc.dram_tensor("mu0", [P, 2 * LW], bf16, kind="ExternalInput")
    s20d = nc.dram_tensor("s20", [P, 2 * LW], bf16, kind="ExternalInput")
    # tick-major bf16 output (includes S*256 garbage cols; host slices)
    outd = nc.dram_tensor("out", [P, NT * 256], bf16, kind="ExternalOutput")

    A_ = sc["ns"] * sc["a_mu"] * (1.0 + 1.0 / sc["nu"])
    C_ = sc["ns"] * sc["a_s"] * (1.0 + 1.0 / sc["nu"])
    D_ = sc["b_s"] - sc["ns"] * sc["a_s"]
    Ct = sc["nu"] * C_
    wt = sc["nu"] * sc["o_s"]

    XR = D_IN - P  # 72 rows of the second x slab
    with tile.TileContext(nc) as tc:
        with (
            tc.tile_pool(name="const", bufs=1) as constp,
            tc.tile_pool(name="big", bufs=1) as bigp,
            tc.tile_pool(name="mlp", bufs=XBUFS) as mlpp,
            tc.tile_pool(name="act", bufs=2) as actp,
            tc.tile_pool(name="scan", bufs=3) as scanp,
            tc.tile_pool(name="psmm", bufs=3, space="PSUM") as psmm,
            tc.tile_pool(name="psdp", bufs=2, space="PSUM") as psdp,
        ):
            # ---- constants on the Scalar DMA queue
            w1a = constp.tile([P, HP], bf16, tag="w1a")
            nc.scalar.dma_start(w1a[:], W1d[0:P, :])
            w1b = constp.tile([XR, HP], bf16, tag="w1b")
            nc.scalar.dma_start(w1b[:], W1d[P:D_IN, :])
            w2 = constp.tile([HP, HP], bf16, tag="w2")
            nc.scalar.dma_start(w2[:], W2d[:])
            w3 = constp.tile([HP, K], bf16, tag="w3")
            nc.scalar.dma_start(w3[:], W3d[:])
            zt = constp.tile([P, 1], f32, tag="zt")
            nc.vector.memset(zt[:], 0.0)

            # ---- persistent tick-interleaved state
            DP = bigp.tile([P, NT * 256], bf16, tag="DP", name="DP")
            MU = bigp.tile([P, (NT + 1) * 256], bf16, tag="MU", name="MU")
            QQ = bigp.tile([P, (NT + 1) * 256], bf16, tag="QQ", name="QQ")

            def dps(t, n=1, half=None):
                a, b = t * 256, (t + n) * 256
                if half == "L":
                    b = a + LW
                elif half == "R":
                    a += LW
                return DP[:, a:b]

            def mus(t, n=1, half=None):
                a, b = t * 256, (t + n) * 256
                if half == "L":
                    b = a + LW
                elif half == "R":
                    a += LW
                return MU[:, a:b]

            def qs(t, n=1, half=None):
                a, b = t * 256, (t + n) * 256
                if half == "L":
                    b = a + LW
                elif half == "R":
                    a += LW
                return QQ[:, a:b]

            # DP viewed [P, tick, 256] for the MLP evac scatter
            DPv = DP[:].rearrange("p (t w) -> p t w", w=256)

            def mlp_chunk(lane, c, relu2_dve=False):
                c_glob = lane * CPL + c
                col0 = c_glob * CHUNK
                xa = mlpp.tile([P, CHUNK], bf16, tag="xa")
                nc.sync.dma_start(xa[:], xT[0:P, col0:col0 + CHUNK])
                xb = mlpp.tile([XR, CHUNK], bf16, tag="xb")
                nc.sync.dma_start(xb[:], xT[P:D_IN, col0:col0 + CHUNK])

                r1 = actp.tile([HP, CHUNK], bf16, tag="r1")
                r2 = actp.tile([HP, CHUNK], bf16, tag="r2")

                ps1 = psmm.tile([HP, CHUNK], f32, tag="mm")
                for j in range(CHUNK // MM_N):
                    s = slice(j * MM_N, (j + 1) * MM_N)
                    nc.tensor.matmul(ps1[:, s], w1a[:], xa[:, s],
                                     start=True, stop=False)
                for j in range(CHUNK // MM_N):
                    s = slice(j * MM_N, (j + 1) * MM_N)
                    nc.tensor.matmul(ps1[:, s], w1b[:], xb[:, s],
                                     start=False, stop=True)
                nc.scalar.activation(r1[:], ps1[:], AF.Relu,
                                     bias=zt[:, 0:1])

                ps2 = psmm.tile([HP, CHUNK], f32, tag="mm")
                for j in range(CHUNK // MM_N):
                    s = slice(j * MM_N, (j + 1) * MM_N)
                    nc.tensor.matmul(ps2[:, s], w2[:], r1[:, s],
                                     start=True, stop=True)
                if relu2_dve:
                    nc.vector.tensor_scalar_max(r2[:], ps2[:], 0.0)
                else:
                    nc.scalar.activation(r2[:], ps2[:], AF.Relu,
                                         bias=zt[:, 0:1])

                psd = psdp.tile([P, (CHUNK // P) * K], f32, tag="dp")
                for j in range(CHUNK // P):
                    nc.tensor.matmul(psd[:, j * K:(j + 1) * K],
                                     r2[:, j * P:(j + 1) * P], w3[:],
                                     start=True, stop=True)
                # evac: dp[tick(k,lane), half + c*8 + j] = psd[:, j*K+k]
                src = psd[:].rearrange("p (j k) -> p k j", k=K)
                t0 = 0 if lane == 0 else S
                off = 0 if lane == 0 else LW
                dst = DPv[:, t0:t0 + K, off + c * 8:off + (c + 1) * 8]
                if EVAC == "pool":
                    nc.gpsimd.tensor_scalar_mul(dst, src, 1.0)
                else:
                    nc.scalar.copy(dst, src)

            def scan_tick(t):
                if t < S:
                    half, w = "L", LW
                elif t < K:
                    half, w = None, 256
                else:
                    half, w = "R", LW
                y = dps(t, half=half)
                mu_p, mu_n = mus(t, half=half), mus(t + 1, half=half)
                q_p, q_n = qs(t, half=half), qs(t + 1, half=half)
                et = scanp.tile([P, 256], bf16, tag="e", name="et")
                ftt = scanp.tile([P, 256], bf16, tag="f", name="ftt")
                m1t = scanp.tile([P, 256], bf16, tag="m", name="m1t")
                e, ft, m1 = et[:, :w], ftt[:, :w], m1t[:, :w]
                nc.vector.tensor_tensor(e, y, mu_p, ALU.subtract)
                nc.vector._custom_dve(cust["ARGAS_FQR"], out=ft, in0=e,
                                      in1=q_p, s0=_RC0, s1=_RC1)
                nc.vector.tensor_tensor(m1, e, ft, ALU.mult)
                nc.vector._custom_dve(cust["ARGAS_AFF_AFF2"], out=mu_n,
                                      in0=mu_p, in1=m1,
                                      s0=A_, s1=sc["o_mu"], imm2=sc["b_mu"])
                nc.vector._custom_dve(cust["ARGAS_QF"], out=q_n,
                                      in0=ft, in1=q_p,
                                      s0=-Ct, s1=Ct + D_, imm2=wt)

            def tail_bulk(t0, n, half=None):
                """out(t) = dp(t)*sqrt(Q(t+1)/nu) + mu(t+1), ticks [t0,t0+n).
                In-place: sg over Q(t+1..); out over dp(t..). Emit only
                after scan tick t0+n (WAR on Q) has been emitted."""
                sg = qs(t0 + 1, n, half=half)
                nc.scalar.activation(sg, sg, AF.Sqrt,
                                     bias=zt[:, 0:1], scale=1.0 / sc["nu"])
                dk = dps(t0, n, half=half)
                eng = nc.gpsimd if TAILENG == "pool" else nc.vector
                eng.tensor_tensor(dk, dk, sg, ALU.mult)
                eng.tensor_tensor(dk, dk, mus(t0 + 1, n, half=half),
                                  ALU.add)

            def dma_out(t0, n):
                nc.sync.dma_start(outd[:, t0 * 256:(t0 + n) * 256],
                                  dps(t0, n))

            # ---------------- emission schedule ----------------
            # init state: lane0 -> MU/QQ slab 0 left; lane1 -> slab S right
            nc.scalar.dma_start(mus(0, half="L"), mu0d[:, 0:LW])
            nc.scalar.dma_start(qs(0, half="L"), s20d[:, 0:LW])
            nc.scalar.dma_start(mus(S, half="R"), mu0d[:, LW:2 * LW])
            nc.scalar.dma_start(qs(S, half="R"), s20d[:, LW:2 * LW])

            for c in range(CPL):          # lane0 MLP (relu2 on idle DVE)
                mlp_chunk(0, c, relu2_dve=(RELU2 == "dve0"))
            for c in range(CPL):          # lane1 MLP (engines self-pace)
                mlp_chunk(1, c)

            # scan ticks + pipelined tails (sqrt ACT, mult/add Pool).
            # Emit tail for ticks [p, p+8) once scan tick p+8 is emitted
            # (WAR: the in-place sqrt overwrites Q slabs the scan reads).
            pend = 0
            for t in range(NT):
                scan_tick(t)
                while pend + TAILB <= t:
                    n = TAILB
                    tail_bulk(pend, n)
                    dma_out(pend, n)
                    pend += n
            # remaining tails (scan fully emitted; WAR safe)
            while pend < NT:
                n = min(TAILB, NT - pend)
                tail_bulk(pend, n)
                dma_out(pend, n)
                pend += n
    if not nc.is_finalized():
        nc.finalize()
    return nc


# ---------------------------------------------------------------- tracing
def _maybe_enable_trace():
    if os.environ.get("BASS_TRACE") != "1":
        return
    try:
        import sys, types
        try:
            import antenv.axon_hooks as ah
        except ImportError:
            import antenv
            ah = types.ModuleType("antenv.axon_hooks")
            ah._hook = None
            def _set(h):
                ah._hook = h
            def _get():
                return ah._hook
            ah.set_axon_ntff_profile_hook = _set
            ah.get_axon_ntff_profile_hook = _get
            sys.modules["antenv.axon_hooks"] = ah
            antenv.axon_hooks = ah
        if ah.get_axon_ntff_profile_hook() is not None:
            return
        from trn_agent_boot.trn_boot import _ntff_profile_via_ctypes
        import concourse.bass_utils as bu
        bu.upload_artifacts = lambda tmpdir: tmpdir
        ah.set_axon_ntff_profile_hook(
            _ntff_profile_via_ctypes("/opt/axon/libaxon_pjrt.so"))
        print("[kernel] NTFF profile hook installed")
    except Exception as e:
        print(f"[kernel] trace hook unavailable: {e}")


LAST = None  # last BassKernelResults (dev/tracing)


# ---------------------------------------------------------------- entry
def kernel(**inputs):
    import ml_dtypes
    bfl = ml_dtypes.bfloat16
    _maybe_enable_trace()
    x = np.asarray(inputs["x"], np.float32)
    last_mu = np.asarray(inputs["last_mu"], np.float32)
    last_sigma = np.asarray(inputs["last_sigma"], np.float32)
    sc = dict(
        a_mu=float(inputs["alpha_mu"]), a_s=float(inputs["alpha_sigma"]),
        b_mu=float(inputs["beta_mu"]), b_s=float(inputs["beta_sigma"]),
        o_mu=float(inputs["omega_mu"]), o_s=float(inputs["omega_sigma"]),
        nu=float(inputs["nu"]), ns=float(inputs["norm_strength"]),
    )
    # biases are structurally zero in this net (setup_inputs); the padded
    # no-bias-row layout depends on it.
    for bn in ("b1", "b2", "b3"):
        assert float(np.abs(np.asarray(inputs[bn])).max()) == 0.0, \
            f"{bn} != 0 unsupported by padded kernel"

    def pad(w, rows, cols):
        out = np.zeros((rows, cols), np.float32)
        a = np.asarray(w, np.float32)
        out[:a.shape[0], :a.shape[1]] = a
        return out.astype(bfl)

    W1e = pad(inputs["W1"], D_IN, HP)
    W2e = pad(inputs["W2"], HP, HP)
    W3e = pad(inputs["W3"], HP, K)

    nc = build_nc(sc)
    in_maps = []
    for cidx in range(NCORES):
        sl = slice(cidx * BC, (cidx + 1) * BC)
        # lane inits: [P, LW] each, col = row-tile within lane
        lm = last_mu[sl].reshape(2, LW, P)          # [lane, tile, p]
        ls = (sc["nu"] * last_sigma[sl]).reshape(2, LW, P)
        mu0 = np.concatenate([lm[0].T, lm[1].T], axis=1)   # [P, 2*LW]
        s20 = np.concatenate([ls[0].T, ls[1].T], axis=1)
        in_maps.append({
            "xT": np.ascontiguousarray(x[sl].T).astype(bfl),
            "W1e": W1e, "W2e": W2e, "W3e": W3e,
            "mu0": np.ascontiguousarray(mu0).astype(bfl),
            "s20": np.ascontiguousarray(s20).astype(bfl),
        })
    res = run_bass_kernel_spmd(nc, in_maps, list(range(NCORES)))
    global LAST
    LAST = res
    if res.exec_time_ns is not None:
        print(f"HW exec time: {res.exec_time_ns} ns")
    # out[p, t*256 + half*LW + c] -> full[(half*LW+c)*P + p, k]
    parts = []
    for i in range(NCORES):
        o = np.asarray(res.results[i]["out"]).astype(np.float32)
        o = o.reshape(P, NT, 2, LW)
        l0 = o[:, 0:K, 0, :].transpose(2, 0, 1).reshape(LW * P, K)
        l1 = o[:, S:S + K, 1, :].transpose(2, 0, 1).reshape(LW * P, K)
        parts.append(l0)
        parts.append(l1)
    return np.concatenate(parts, 0)


# revision 11
# speedup vs baseline: 1.1991x; 1.1991x over previous
"""AR-GAS-Net Trainium2 kernel v4 (8-core SPMD, data-parallel over batch).

Per core (BC=32768 rows, 256 row-tiles of 128):
  - bf16 MLP on TensorE (as v3): H padded 100->128, zero-bias net,
    x host-cast bf16, CHUNK=1024-row chunks.
  - 64-step GAS scan, 2 lanes of 128 row-tiles with skew S in a
    tick-interleaved layout: tick t's slab is a CONTIGUOUS [128, 256]
    region whose left half is lane0@k=t and right half is lane1@k=t-S.
    All scan ops are packed 2D APs (3D strided APs run ~3.5x slower on
    the DVE: 716ns vs 202ns per 256 cols, measured).
  - chain per tick (all DVE, no cross-engine hops):
      e   = dp - mu                  TT   (2x_1p, ~202ns @256c)
      f   = Q * recip1NR(e^2 + Q)    custom FQR  (1x, ~332ns; 8/8 uop
                                     stages, +-0.17% recip err)
      m1  = e * f                    TT   (~202ns)
      mu' = (A*m1 + o_mu) + b_mu*mu  custom AFF_AFF2
      Q'  = (-Ct*f + Ct+D)*Q + wt    custom QF
  - tails (sg=sqrt(Q'/nu); out = dp*sg + mu') bulk-pipelined: sqrt on
    ACT, mult/add on DVE over packed multi-tick regions.
  - MLP relus on ACT; dp-evac copies on Pool; x DMA on Sync queue.
"""

import os
import numpy as np

import concourse.bass as bass
import concourse.bacc as bacc
import concourse.mybir as mybir
from concourse import tile
from concourse.bass_utils import run_bass_kernel_spmd

f32 = mybir.dt.float32
bf16 = mybir.dt.bfloat16
AF = mybir.ActivationFunctionType
ALU = mybir.AluOpType

B, K, D_IN, H = 262144, 64, 200, 100
HP = 128                    # padded hidden width (zero-bias net)
NCORES = 8
BC = B // NCORES            # 32768 rows per core
P = 128
T = BC // P                 # 256 row-tiles
LW = 128                    # lane width (row-tiles per lane)
S = int(os.environ.get("ARGAS_S", "32"))   # lane skew in ticks
NT = K + S                  # number of ticks
CHUNK = 1024                # MLP chunk rows
NCH = BC // CHUNK           # 32 chunks
CPL = NCH // 2              # chunks per lane (16)
XBUFS = int(os.environ.get("ARGAS_XBUFS", "6"))
MM_N = int(os.environ.get("ARGAS_MMN", "512"))
EVAC = os.environ.get("ARGAS_EVAC", "act")       # act|pool (pool can't
                                                 # read PSUM -> act only)
RELU2 = os.environ.get("ARGAS_RELU2", "dve0")    # act|dve0 (lane0 on DVE)
TAILENG = os.environ.get("ARGAS_TAILENG", "dve")   # dve|pool (pool SBUF
                                                   # traffic slows DVE 2x)
TAILLAG = int(os.environ.get("ARGAS_TAILLAG", "6"))  # ticks between sqrt
                                                     # and mult/add
TAILB = int(os.environ.get("ARGAS_TAILB", "8"))  # tail bulk ticks

# recip seed constants (optimal for the 1-NR variant too; see dve_ops)
_RC0, _RC1 = -0.23549792, 2.0017324

# ---------------------------------------------------------------- custom ops
_CUSTOM = None


def _register_custom_ops():
    global _CUSTOM
    if _CUSTOM is not None:
        return _CUSTOM
    import concourse.dve_ops as dve_ops
    from concourse.dve_spec import (
        Spec, Src0, Src1, C0, C1, C2, sq, lower, Bin, AluOp)
    from concourse.dve_uop import DveOpSpec

    def _ref_fqr(in0, in1, c0, c1, c2):
        d = (in0.astype(np.float32) ** 2 + in1).astype(np.float32)
        nx = (~d.view(np.int32)).view(np.float32)
        y0 = nx * np.float32(c0)
        y1 = (y0 * (np.float32(c1) - d * y0)).astype(np.float32)
        return in1 * y1

    _d = sq(Src0) + Src1
    _nx = Bin(AluOp.BITWISE_NOT, _d, _d)
    _y0 = _nx * C0
    _y1 = _y0 * (C1 - _d * _y0)

    defs = [
        # f = Q * recip1NR(e*e + Q); Src0=e, Src1=Q
        ("ARGAS_FQR", Spec(
            body=_y1 * Src1,
            reference=_ref_fqr)),
        # mu' = (m1*A + o_mu) + mu*b_mu  (Src0=mu, Src1=m1)
        ("ARGAS_AFF_AFF2", Spec(
            body=(Src1 * C0 + C1) + Src0 * C2,
            reference=lambda in0, in1, c0, c1, c2:
                (in1.astype(np.float32) * c0 + c1) + in0 * c2)),
        # Q' = ((f*C0 + C1))*Q + C2   (Src0=f, Src1=Q)
        ("ARGAS_QF", Spec(
            body=(Src0 * C0 + C1) * Src1 + C2,
            reference=lambda in0, in1, c0, c1, c2:
                (in0.astype(np.float32) * c0 + c1) * in1 + c2)),
    ]
    ops = {}
    for name, spec in defs:
        if name not in dve_ops._SUB_OPCODE_FOR_NAME:
            row = dve_ops._CUSTOM_DVE_ROW_BASE + len(dve_ops.OPS)
            assert row < 0x20, "custom-DVE row overflow"
            dve_ops._SUB_OPCODE_FOR_NAME[name] = row
        tmp = {}
        for ver in ("v3", "v4"):
            try:
                s = DveOpSpec(
                    name=name,
                    opcode=dve_ops.get_dve_sub_opcode(name),
                    uops=lower(spec, ver=ver),
                    rd1_en=True,
                )
                tmp[ver] = s.sha(ver)
            except Exception:
                pass
        op = dve_ops.DveOp(name, spec, subdim=False, uops_sha=tmp)
        if all(o.name != name for o in dve_ops.OPS):
            dve_ops.OPS.append(op)
        dve_ops.CUSTOM_DVE_SPECS[name] = spec
        ops[name] = op
    _CUSTOM = ops
    return _CUSTOM


# ---------------------------------------------------------------- builder
def build_nc(sc):
    cust = _register_custom_ops()
    nc = bacc.Bacc(None)

    xT = nc.dram_tensor("xT", [D_IN, BC], bf16, kind="ExternalInput")
    W1d = nc.dram_tensor("W1e", [D_IN, HP], bf16, kind="ExternalInput")
    W2d = nc.dram_tensor("W2e", [HP, HP], bf16, kind="ExternalInput")
    W3d = nc.dram_tensor("W3e", [HP, K], bf16, kind="ExternalInput")
    # per-lane init state, [P, LW] each
    mu0d = nc.dram_tensor("mu0", [P, 2 * LW], bf16, kind="ExternalInput")
    s20d = nc.dram_tensor("s20", [P, 2 * LW], bf16, kind="ExternalInput")
    # tick-major bf16 output (includes S*256 garbage cols; host slices)
    outd = nc.dram_tensor("out", [P, NT * 256], bf16, kind="ExternalOutput")

    A_ = sc["ns"] * sc["a_mu"] * (1.0 + 1.0 / sc["nu"])
    C_ = sc["ns"] * sc["a_s"] * (1.0 + 1.0 / sc["nu"])
    D_ = sc["b_s"] - sc["ns"] * sc["a_s"]
    Ct = sc["nu"] * C_
    wt = sc["nu"] * sc["o_s"]

    XR = D_IN - P  # 72 rows of the second x slab
    with tile.TileContext(nc) as tc:
        with (
            tc.tile_pool(name="const", bufs=1) as constp,
            tc.tile_pool(name="big", bufs=1) as bigp,
            tc.tile_pool(name="mlp", bufs=XBUFS) as mlpp,
            tc.tile_pool(name="act", bufs=2) as actp,
            tc.tile_pool(name="scan", bufs=3) as scanp,
            tc.tile_pool(name="psmm", bufs=3, space="PSUM") as psmm,
            tc.tile_pool(name="psdp", bufs=2, space="PSUM") as psdp,
        ):
            # ---- constants on the Scalar DMA queue
            w1a = constp.tile([P, HP], bf16, tag="w1a")
            nc.scalar.dma_start(w1a[:], W1d[0:P, :])
            w1b = constp.tile([XR, HP], bf16, tag="w1b")
            nc.scalar.dma_start(w1b[:], W1d[P:D_IN, :])
            w2 = constp.tile([HP, HP], bf16, tag="w2")
            nc.scalar.dma_start(w2[:], W2d[:])
            w3 = constp.tile([HP, K], bf16, tag="w3")
            nc.scalar.dma_start(w3[:], W3d[:])
            zt = constp.tile([P, 1], f32, tag="zt")
            nc.vector.memset(zt[:], 0.0)

            # ---- persistent tick-interleaved state
            DP = bigp.tile([P, NT * 256], bf16, tag="DP", name="DP")
            MU = bigp.tile([P, (NT + 1) * 256], bf16, tag="MU", name="MU")
            QQ = bigp.tile([P, (NT + 1) * 256], bf16, tag="QQ", name="QQ")

            def dps(t, n=1, half=None):
                a, b = t * 256, (t + n) * 256
                if half == "L":
                    b = a + LW
                elif half == "R":
                    a += LW
                return DP[:, a:b]

            def mus(t, n=1, half=None):
                a, b = t * 256, (t + n) * 256
                if half == "L":
                    b = a + LW
                elif half == "R":
                    a += LW
                return MU[:, a:b]

            def qs(t, n=1, half=None):
                a, b = t * 256, (t + n) * 256
                if half == "L":
                    b = a + LW
                elif half == "R":
                    a += LW
                return QQ[:, a:b]

            # DP viewed [P, tick, 256] for the MLP evac scatter
            DPv = DP[:].rearrange("p (t w) -> p t w", w=256)

            def mlp_chunk(lane, c, relu2_dve=False):
                c_glob = lane * CPL + c
                col0 = c_glob * CHUNK
                xa = mlpp.tile([P, CHUNK], bf16, tag="xa")
                nc.sync.dma_start(xa[:], xT[0:P, col0:col0 + CHUNK])
                xb = mlpp.tile([XR, CHUNK], bf16, tag="xb")
                nc.sync.dma_start(xb[:], xT[P:D_IN, col0:col0 + CHUNK])

                r1 = actp.tile([HP, CHUNK], bf16, tag="r1")
                r2 = actp.tile([HP, CHUNK], bf16, tag="r2")

                ps1 = psmm.tile([HP, CHUNK], f32, tag="mm")
                for j in range(CHUNK // MM_N):
                    s = slice(j * MM_N, (j + 1) * MM_N)
                    nc.tensor.matmul(ps1[:, s], w1a[:], xa[:, s],
                                     start=True, stop=False)
                for j in range(CHUNK // MM_N):
                    s = slice(j * MM_N, (j + 1) * MM_N)
                    nc.tensor.matmul(ps1[:, s], w1b[:], xb[:, s],
                                     start=False, stop=True)
                nc.scalar.activation(r1[:], ps1[:], AF.Relu,
                                     bias=zt[:, 0:1])

                ps2 = psmm.tile([HP, CHUNK], f32, tag="mm")
                for j in range(CHUNK // MM_N):
                    s = slice(j * MM_N, (j + 1) * MM_N)
                    nc.tensor.matmul(ps2[:, s], w2[:], r1[:, s],
                                     start=True, stop=True)
                if relu2_dve:
                    nc.vector.tensor_scalar_max(r2[:], ps2[:], 0.0)
                else:
                    nc.scalar.activation(r2[:], ps2[:], AF.Relu,
                                         bias=zt[:, 0:1])

                psd = psdp.tile([P, (CHUNK // P) * K], f32, tag="dp")
                for j in range(CHUNK // P):
                    nc.tensor.matmul(psd[:, j * K:(j + 1) * K],
                                     r2[:, j * P:(j + 1) * P], w3[:],
                                     start=True, stop=True)
                # evac: dp[tick(k,lane), half + c*8 + j] = psd[:, j*K+k]
                src = psd[:].rearrange("p (j k) -> p k j", k=K)
                t0 = 0 if lane == 0 else S
                off = 0 if lane == 0 else LW
                dst = DPv[:, t0:t0 + K, off + c * 8:off + (c + 1) * 8]
                if EVAC == "pool":
                    nc.gpsimd.tensor_scalar_mul(dst, src, 1.0)
                else:
                    nc.scalar.copy(dst, src)

            def scan_tick(t):
                if t < S:
                    half, w = "L", LW
                elif t < K:
                    half, w = None, 256
                else:
                    half, w = "R", LW
                y = dps(t, half=half)
                mu_p, mu_n = mus(t, half=half), mus(t + 1, half=half)
                q_p, q_n = qs(t, half=half), qs(t + 1, half=half)
                et = scanp.tile([P, 256], bf16, tag="e", name="et")
                ftt = scanp.tile([P, 256], bf16, tag="f", name="ftt")
                m1t = scanp.tile([P, 256], bf16, tag="m", name="m1t")
                e, ft, m1 = et[:, :w], ftt[:, :w], m1t[:, :w]
                nc.vector.tensor_tensor(e, y, mu_p, ALU.subtract)
                nc.vector._custom_dve(cust["ARGAS_FQR"], out=ft, in0=e,
                                      in1=q_p, s0=_RC0, s1=_RC1)
                nc.vector.tensor_tensor(m1, e, ft, ALU.mult)
                nc.vector._custom_dve(cust["ARGAS_AFF_AFF2"], out=mu_n,
                                      in0=mu_p, in1=m1,
                                      s0=A_, s1=sc["o_mu"], imm2=sc["b_mu"])
                nc.vector._custom_dve(cust["ARGAS_QF"], out=q_n,
                                      in0=ft, in1=q_p,
                                      s0=-Ct, s1=Ct + D_, imm2=wt)

            def tail_sqrt(t0, n):
                """sg = sqrt(Q(t+1)/nu) in place, ticks [t0,t0+n). Emit only
                after scan tick t0+n (WAR on Q) has been emitted."""
                sg = qs(t0 + 1, n)
                nc.scalar.activation(sg, sg, AF.Sqrt,
                                     bias=zt[:, 0:1], scale=1.0 / sc["nu"])

            def tail_ma(t0, n):
                """out(t) = dp(t)*sg(t+1) + mu(t+1) in place over dp."""
                dk = dps(t0, n)
                eng = nc.gpsimd if TAILENG == "pool" else nc.vector
                eng.tensor_tensor(dk, dk, qs(t0 + 1, n), ALU.mult)
                eng.tensor_tensor(dk, dk, mus(t0 + 1, n), ALU.add)

            def dma_out(t0, n):
                nc.sync.dma_start(outd[:, t0 * 256:(t0 + n) * 256],
                                  dps(t0, n))

            # ---------------- emission schedule ----------------
            # init state: lane0 -> MU/QQ slab 0 left; lane1 -> slab S right
            nc.scalar.dma_start(mus(0, half="L"), mu0d[:, 0:LW])
            nc.scalar.dma_start(qs(0, half="L"), s20d[:, 0:LW])
            nc.scalar.dma_start(mus(S, half="R"), mu0d[:, LW:2 * LW])
            nc.scalar.dma_start(qs(S, half="R"), s20d[:, LW:2 * LW])

            for c in range(CPL):          # lane0 MLP (relu2 on idle DVE)
                mlp_chunk(0, c, relu2_dve=(RELU2 == "dve0"))
            for c in range(CPL):          # lane1 MLP (engines self-pace)
                mlp_chunk(1, c)

            # scan ticks + pipelined tails.  sqrt (ACT) for a bulk is
            # emitted once scan tick t0+TAILB exists (WAR on Q); the DVE
            # mult/add trail TAILLAG ticks further so the DVE queue never
            # waits on the ACT sqrt.
            psq = pma = 0
            for t in range(NT):
                scan_tick(t)
                if psq + TAILB <= t:
                    tail_sqrt(psq, TAILB)
                    psq += TAILB
                if pma + TAILB <= t - TAILLAG:
                    tail_ma(pma, TAILB)
                    dma_out(pma, TAILB)
                    pma += TAILB
            while psq < NT:
                n = min(TAILB, NT - psq)
                tail_sqrt(psq, n)
                psq += n
            while pma < NT:
                n = min(TAILB, NT - pma)
                tail_ma(pma, n)
                dma_out(pma, n)
                pma += n
    if not nc.is_finalized():
        nc.finalize()
    return nc


# ---------------------------------------------------------------- tracing
def _maybe_enable_trace():
    if os.environ.get("BASS_TRACE") != "1":
        return
    try:
        import sys, types
        try:
            import antenv.axon_hooks as ah
        except ImportError:
            import antenv
            ah = types.ModuleType("antenv.axon_hooks")
            ah._hook = None
            def _set(h):
                ah._hook = h
            def _get():
                return ah._hook
            ah.set_axon_ntff_profile_hook = _set
            ah.get_axon_ntff_profile_hook = _get
            sys.modules["antenv.axon_hooks"] = ah
            antenv.axon_hooks = ah
        if ah.get_axon_ntff_profile_hook() is not None:
            return
        from trn_agent_boot.trn_boot import _ntff_profile_via_ctypes
        import concourse.bass_utils as bu
        bu.upload_artifacts = lambda tmpdir: tmpdir
        ah.set_axon_ntff_profile_hook(
            _ntff_profile_via_ctypes("/opt/axon/libaxon_pjrt.so"))
        print("[kernel] NTFF profile hook installed")
    except Exception as e:
        print(f"[kernel] trace hook unavailable: {e}")


LAST = None  # last BassKernelResults (dev/tracing)


# ---------------------------------------------------------------- entry
def kernel(**inputs):
    import ml_dtypes
    bfl = ml_dtypes.bfloat16
    _maybe_enable_trace()
    x = np.asarray(inputs["x"], np.float32)
    last_mu = np.asarray(inputs["last_mu"], np.float32)
    last_sigma = np.asarray(inputs["last_sigma"], np.float32)
    sc = dict(
        a_mu=float(inputs["alpha_mu"]), a_s=float(inputs["alpha_sigma"]),
        b_mu=float(inputs["beta_mu"]), b_s=float(inputs["beta_sigma"]),
        o_mu=float(inputs["omega_mu"]), o_s=float(inputs["omega_sigma"]),
        nu=float(inputs["nu"]), ns=float(inputs["norm_strength"]),
    )
    # biases are structurally zero in this net (setup_inputs); the padded
    # no-bias-row layout depends on it.
    for bn in ("b1", "b2", "b3"):
        assert float(np.abs(np.asarray(inputs[bn])).max()) == 0.0, \
            f"{bn} != 0 unsupported by padded kernel"

    def pad(w, rows, cols):
        out = np.zeros((rows, cols), np.float32)
        a = np.asarray(w, np.float32)
        out[:a.shape[0], :a.shape[1]] = a
        return out.astype(bfl)

    W1e = pad(inputs["W1"], D_IN, HP)
    W2e = pad(inputs["W2"], HP, HP)
    W3e = pad(inputs["W3"], HP, K)

    nc = build_nc(sc)
    in_maps = []
    for cidx in range(NCORES):
        sl = slice(cidx * BC, (cidx + 1) * BC)
        # lane inits: [P, LW] each, col = row-tile within lane
        lm = last_mu[sl].reshape(2, LW, P)          # [lane, tile, p]
        ls = (sc["nu"] * last_sigma[sl]).reshape(2, LW, P)
        mu0 = np.concatenate([lm[0].T, lm[1].T], axis=1)   # [P, 2*LW]
        s20 = np.concatenate([ls[0].T, ls[1].T], axis=1)
        in_maps.append({
            "xT": np.ascontiguousarray(x[sl].T).astype(bfl),
            "W1e": W1e, "W2e": W2e, "W3e": W3e,
            "mu0": np.ascontiguousarray(mu0).astype(bfl),
            "s20": np.ascontiguousarray(s20).astype(bfl),
        })
    res = run_bass_kernel_spmd(nc, in_maps, list(range(NCORES)))
    global LAST
    LAST = res
    if res.exec_time_ns is not None:
        print(f"HW exec time: {res.exec_time_ns} ns")
    # out[p, t*256 + half*LW + c] -> full[(half*LW+c)*P + p, k]
    parts = []
    for i in range(NCORES):
        o = np.asarray(res.results[i]["out"]).astype(np.float32)
        o = o.reshape(P, NT, 2, LW)
        l0 = o[:, 0:K, 0, :].transpose(2, 0, 1).reshape(LW * P, K)
        l1 = o[:, S:S + K, 1, :].transpose(2, 0, 1).reshape(LW * P, K)
        parts.append(l0)
        parts.append(l1)
    return np.concatenate(parts, 0)


# revision 13
# speedup vs baseline: 1.3829x; 1.1533x over previous
"""AR-GAS-Net Trainium2 kernel v4 (8-core SPMD, data-parallel over batch).

Per core (BC=32768 rows, 256 row-tiles of 128):
  - bf16 MLP on TensorE (as v3): H padded 100->128, zero-bias net,
    x host-cast bf16, CHUNK=1024-row chunks.
  - 64-step GAS scan, 2 lanes of 128 row-tiles with skew S in a
    tick-interleaved layout: tick t's slab is a CONTIGUOUS [128, 256]
    region whose left half is lane0@k=t and right half is lane1@k=t-S.
    All scan ops are packed 2D APs (3D strided APs run ~3.5x slower on
    the DVE: 716ns vs 202ns per 256 cols, measured).
  - chain per tick (all DVE, no cross-engine hops):
      e   = dp - mu                  TT   (2x_1p, ~202ns @256c)
      f   = Q * recip1NR(e^2 + Q)    custom FQR  (1x, ~332ns; 8/8 uop
                                     stages, +-0.17% recip err)
      m1  = e * f                    TT   (~202ns)
      mu' = (A*m1 + o_mu) + b_mu*mu  custom AFF_AFF2
      Q'  = (-Ct*f + Ct+D)*Q + wt    custom QF
  - tails (sg=sqrt(Q'/nu); out = dp*sg + mu') bulk-pipelined: sqrt on
    ACT, mult/add on DVE over packed multi-tick regions.
  - MLP relus on ACT; dp-evac copies on Pool; x DMA on Sync queue.
"""

import os
import numpy as np

import concourse.bass as bass
import concourse.bacc as bacc
import concourse.mybir as mybir
from concourse import tile
from concourse.bass_utils import run_bass_kernel_spmd

f32 = mybir.dt.float32
bf16 = mybir.dt.bfloat16
AF = mybir.ActivationFunctionType
ALU = mybir.AluOpType

B, K, D_IN, H = 262144, 64, 200, 100
HP = 128                    # padded hidden width (zero-bias net)
NCORES = 8
BC = B // NCORES            # 32768 rows per core
P = 128
T = BC // P                 # 256 row-tiles
LW = 128                    # lane width (row-tiles per lane)
S = int(os.environ.get("ARGAS_S", "32"))   # lane skew in ticks
NT = K + S                  # number of ticks
CHUNK = 1024                # MLP chunk rows
NCH = BC // CHUNK           # 32 chunks
CPL = NCH // 2              # chunks per lane (16)
XBUFS = int(os.environ.get("ARGAS_XBUFS", "6"))
MM_N = int(os.environ.get("ARGAS_MMN", "512"))
EVAC = os.environ.get("ARGAS_EVAC", "act")       # act|pool (pool can't
                                                 # read PSUM -> act only)
RELU2 = os.environ.get("ARGAS_RELU2", "dve0")    # act|dve0 (lane0 on DVE)
TAILENG = os.environ.get("ARGAS_TAILENG", "dve")   # dve|pool (pool SBUF
                                                   # traffic slows DVE 2x)
TAILLAG = int(os.environ.get("ARGAS_TAILLAG", "6"))  # ticks between sqrt
                                                     # and mult/add
TAILB = int(os.environ.get("ARGAS_TAILB", "8"))  # tail bulk ticks

# recip seed constants (optimal for the 1-NR variant too; see dve_ops)
_RC0, _RC1 = -0.23549792, 2.0017324

# ---------------------------------------------------------------- custom ops
_CUSTOM = None


def _register_custom_ops():
    global _CUSTOM
    if _CUSTOM is not None:
        return _CUSTOM
    import concourse.dve_ops as dve_ops
    from concourse.dve_spec import (
        Spec, Src0, Src1, C0, C1, C2, sq, lower, Bin, AluOp)
    from concourse.dve_uop import DveOpSpec

    def _ref_fqr(in0, in1, c0, c1, c2):
        d = (in0.astype(np.float32) ** 2 + in1).astype(np.float32)
        nx = (~d.view(np.int32)).view(np.float32)
        y0 = nx * np.float32(c0)
        y1 = (y0 * (np.float32(c1) - d * y0)).astype(np.float32)
        return in1 * y1

    _d = sq(Src0) + Src1
    _nx = Bin(AluOp.BITWISE_NOT, _d, _d)
    _y0 = _nx * C0
    _y1 = _y0 * (C1 - _d * _y0)

    defs = [
        # f = Q * recip1NR(e*e + Q); Src0=e, Src1=Q
        ("ARGAS_FQR", Spec(
            body=_y1 * Src1,
            reference=_ref_fqr)),
        # mu' = (m1*A + o_mu) + mu*b_mu  (Src0=mu, Src1=m1)
        ("ARGAS_AFF_AFF2", Spec(
            body=(Src1 * C0 + C1) + Src0 * C2,
            reference=lambda in0, in1, c0, c1, c2:
                (in1.astype(np.float32) * c0 + c1) + in0 * c2)),
        # Q' = ((f*C0 + C1))*Q + C2   (Src0=f, Src1=Q)
        ("ARGAS_QF", Spec(
            body=(Src0 * C0 + C1) * Src1 + C2,
            reference=lambda in0, in1, c0, c1, c2:
                (in0.astype(np.float32) * c0 + c1) * in1 + c2)),
    ]
    ops = {}
    for name, spec in defs:
        if name not in dve_ops._SUB_OPCODE_FOR_NAME:
            row = dve_ops._CUSTOM_DVE_ROW_BASE + len(dve_ops.OPS)
            assert row < 0x20, "custom-DVE row overflow"
            dve_ops._SUB_OPCODE_FOR_NAME[name] = row
        tmp = {}
        for ver in ("v3", "v4"):
            try:
                s = DveOpSpec(
                    name=name,
                    opcode=dve_ops.get_dve_sub_opcode(name),
                    uops=lower(spec, ver=ver),
                    rd1_en=True,
                )
                tmp[ver] = s.sha(ver)
            except Exception:
                pass
        op = dve_ops.DveOp(name, spec, subdim=False, uops_sha=tmp)
        if all(o.name != name for o in dve_ops.OPS):
            dve_ops.OPS.append(op)
        dve_ops.CUSTOM_DVE_SPECS[name] = spec
        ops[name] = op
    _CUSTOM = ops
    return _CUSTOM


# ---------------------------------------------------------------- builder
def build_nc(sc):
    cust = _register_custom_ops()
    nc = bacc.Bacc(None)

    xT = nc.dram_tensor("xT", [D_IN, BC], bf16, kind="ExternalInput")
    W1d = nc.dram_tensor("W1e", [D_IN, HP], bf16, kind="ExternalInput")
    W2d = nc.dram_tensor("W2e", [HP, HP], bf16, kind="ExternalInput")
    W3d = nc.dram_tensor("W3e", [HP, K], bf16, kind="ExternalInput")
    # per-lane init state, [P, LW] each
    mu0d = nc.dram_tensor("mu0", [P, 2 * LW], bf16, kind="ExternalInput")
    s20d = nc.dram_tensor("s20", [P, 2 * LW], bf16, kind="ExternalInput")
    # tick-major bf16 output (includes S*256 garbage cols; host slices)
    outd = nc.dram_tensor("out", [P, NT * 256], bf16, kind="ExternalOutput")

    A_ = sc["ns"] * sc["a_mu"] * (1.0 + 1.0 / sc["nu"])
    C_ = sc["ns"] * sc["a_s"] * (1.0 + 1.0 / sc["nu"])
    D_ = sc["b_s"] - sc["ns"] * sc["a_s"]
    Ct = sc["nu"] * C_
    wt = sc["nu"] * sc["o_s"]

    XR = D_IN - P  # 72 rows of the second x slab
    with tile.TileContext(nc) as tc:
        with (
            tc.tile_pool(name="const", bufs=1) as constp,
            tc.tile_pool(name="big", bufs=1) as bigp,
            tc.tile_pool(name="mlp", bufs=XBUFS) as mlpp,
            tc.tile_pool(name="act", bufs=2) as actp,
            tc.tile_pool(name="scan", bufs=3) as scanp,
            tc.tile_pool(name="psmm", bufs=3, space="PSUM") as psmm,
            tc.tile_pool(name="psdp", bufs=2, space="PSUM") as psdp,
        ):
            # ---- constants on the Scalar DMA queue
            w1a = constp.tile([P, HP], bf16, tag="w1a")
            nc.scalar.dma_start(w1a[:], W1d[0:P, :])
            w1b = constp.tile([XR, HP], bf16, tag="w1b")
            nc.scalar.dma_start(w1b[:], W1d[P:D_IN, :])
            w2 = constp.tile([HP, HP], bf16, tag="w2")
            nc.scalar.dma_start(w2[:], W2d[:])
            w3 = constp.tile([HP, K], bf16, tag="w3")
            nc.scalar.dma_start(w3[:], W3d[:])
            zt = constp.tile([P, 1], f32, tag="zt")
            nc.vector.memset(zt[:], 0.0)

            # ---- persistent tick-interleaved state
            DP = bigp.tile([P, NT * 256], bf16, tag="DP", name="DP")
            MU = bigp.tile([P, (NT + 1) * 256], bf16, tag="MU", name="MU")
            QQ = bigp.tile([P, (NT + 1) * 256], bf16, tag="QQ", name="QQ")

            def dps(t, n=1, half=None):
                a, b = t * 256, (t + n) * 256
                if half == "L":
                    b = a + LW
                elif half == "R":
                    a += LW
                return DP[:, a:b]

            def mus(t, n=1, half=None):
                a, b = t * 256, (t + n) * 256
                if half == "L":
                    b = a + LW
                elif half == "R":
                    a += LW
                return MU[:, a:b]

            def qs(t, n=1, half=None):
                a, b = t * 256, (t + n) * 256
                if half == "L":
                    b = a + LW
                elif half == "R":
                    a += LW
                return QQ[:, a:b]

            # DP viewed [P, tick, 256] for the MLP evac scatter
            DPv = DP[:].rearrange("p (t w) -> p t w", w=256)

            # --- MLP as software-pipelined stages (PE never waits a relu)
            _st = {}

            def mlpA(i):  # x DMA + L1 matmuls
                col0 = i * CHUNK
                xa = mlpp.tile([P, CHUNK], bf16, tag="xa")
                nc.sync.dma_start(xa[:], xT[0:P, col0:col0 + CHUNK])
                xb = mlpp.tile([XR, CHUNK], bf16, tag="xb")
                nc.sync.dma_start(xb[:], xT[P:D_IN, col0:col0 + CHUNK])
                ps1 = psmm.tile([HP, CHUNK], f32, tag="mm")
                for j in range(CHUNK // MM_N):
                    s = slice(j * MM_N, (j + 1) * MM_N)
                    nc.tensor.matmul(ps1[:, s], w1a[:], xa[:, s],
                                     start=True, stop=False)
                for j in range(CHUNK // MM_N):
                    s = slice(j * MM_N, (j + 1) * MM_N)
                    nc.tensor.matmul(ps1[:, s], w1b[:], xb[:, s],
                                     start=False, stop=True)
                _st[("ps1", i)] = ps1

            def mlpB(i):  # relu1
                r1 = actp.tile([HP, CHUNK], bf16, tag="r1")
                nc.scalar.activation(r1[:], _st.pop(("ps1", i))[:], AF.Relu,
                                     bias=zt[:, 0:1])
                _st[("r1", i)] = r1

            def mlpC(i):  # L2 matmuls
                ps2 = psmm.tile([HP, CHUNK], f32, tag="mm")
                r1 = _st.pop(("r1", i))
                for j in range(CHUNK // MM_N):
                    s = slice(j * MM_N, (j + 1) * MM_N)
                    nc.tensor.matmul(ps2[:, s], w2[:], r1[:, s],
                                     start=True, stop=True)
                _st[("ps2", i)] = ps2

            def mlpD(i, relu2_dve):  # relu2
                r2 = actp.tile([HP, CHUNK], bf16, tag="r2")
                ps2 = _st.pop(("ps2", i))
                if relu2_dve:
                    nc.vector.tensor_scalar_max(r2[:], ps2[:], 0.0)
                else:
                    nc.scalar.activation(r2[:], ps2[:], AF.Relu,
                                         bias=zt[:, 0:1])
                _st[("r2", i)] = r2

            def mlpE(i):  # L3 matmuls + evac
                lane, c = divmod(i, CPL)
                r2 = _st.pop(("r2", i))
                psd = psdp.tile([P, (CHUNK // P) * K], f32, tag="dp")
                for j in range(CHUNK // P):
                    nc.tensor.matmul(psd[:, j * K:(j + 1) * K],
                                     r2[:, j * P:(j + 1) * P], w3[:],
                                     start=True, stop=True)
                # evac: dp[tick(k,lane), half + c*8 + j] = psd[:, j*K+k]
                src = psd[:].rearrange("p (j k) -> p k j", k=K)
                t0 = 0 if lane == 0 else S
                off = 0 if lane == 0 else LW
                dst = DPv[:, t0:t0 + K, off + c * 8:off + (c + 1) * 8]
                nc.scalar.copy(dst, src)

            def mlp_pipelined(relu2_dve_lane0):
                # stage c: A(c+2) B(c+1) C(c+1) D(c) E(c) -- PE sees
                # L1(c+2), L2(c+1), L3(c) back-to-back, one chunk of slack
                # against each relu.
                for i in range(NCH + 2):
                    if i < NCH:
                        mlpA(i)
                    if 1 <= i <= NCH:
                        mlpB(i - 1)
                        mlpC(i - 1)
                    if i >= 2:
                        j = i - 2
                        mlpD(j, relu2_dve_lane0 and j < CPL)
                        mlpE(j)

            def scan_tick(t):
                if t < S:
                    half, w = "L", LW
                elif t < K:
                    half, w = None, 256
                else:
                    half, w = "R", LW
                y = dps(t, half=half)
                mu_p, mu_n = mus(t, half=half), mus(t + 1, half=half)
                q_p, q_n = qs(t, half=half), qs(t + 1, half=half)
                et = scanp.tile([P, 256], bf16, tag="e", name="et")
                ftt = scanp.tile([P, 256], bf16, tag="f", name="ftt")
                m1t = scanp.tile([P, 256], bf16, tag="m", name="m1t")
                e, ft, m1 = et[:, :w], ftt[:, :w], m1t[:, :w]
                nc.vector.tensor_tensor(e, y, mu_p, ALU.subtract)
                nc.vector._custom_dve(cust["ARGAS_FQR"], out=ft, in0=e,
                                      in1=q_p, s0=_RC0, s1=_RC1)
                nc.vector.tensor_tensor(m1, e, ft, ALU.mult)
                nc.vector._custom_dve(cust["ARGAS_AFF_AFF2"], out=mu_n,
                                      in0=mu_p, in1=m1,
                                      s0=A_, s1=sc["o_mu"], imm2=sc["b_mu"])
                nc.vector._custom_dve(cust["ARGAS_QF"], out=q_n,
                                      in0=ft, in1=q_p,
                                      s0=-Ct, s1=Ct + D_, imm2=wt)

            def tail_sqrt(t0, n):
                """sg = sqrt(Q(t+1)/nu) in place, ticks [t0,t0+n). Emit only
                after scan tick t0+n (WAR on Q) has been emitted."""
                sg = qs(t0 + 1, n)
                nc.scalar.activation(sg, sg, AF.Sqrt,
                                     bias=zt[:, 0:1], scale=1.0 / sc["nu"])

            def tail_ma(t0, n):
                """out(t) = dp(t)*sg(t+1) + mu(t+1) in place over dp."""
                dk = dps(t0, n)
                eng = nc.gpsimd if TAILENG == "pool" else nc.vector
                eng.tensor_tensor(dk, dk, qs(t0 + 1, n), ALU.mult)
                eng.tensor_tensor(dk, dk, mus(t0 + 1, n), ALU.add)

            def dma_out(t0, n):
                nc.sync.dma_start(outd[:, t0 * 256:(t0 + n) * 256],
                                  dps(t0, n))

            # ---------------- emission schedule ----------------
            # init state: lane0 -> MU/QQ slab 0 left; lane1 -> slab S right
            nc.scalar.dma_start(mus(0, half="L"), mu0d[:, 0:LW])
            nc.scalar.dma_start(qs(0, half="L"), s20d[:, 0:LW])
            nc.scalar.dma_start(mus(S, half="R"), mu0d[:, LW:2 * LW])
            nc.scalar.dma_start(qs(S, half="R"), s20d[:, LW:2 * LW])

            mlp_pipelined(RELU2 == "dve0")

            # scan ticks + pipelined tails.  sqrt (ACT) for a bulk is
            # emitted once scan tick t0+TAILB exists (WAR on Q); the DVE
            # mult/add trail TAILLAG ticks further so the DVE queue never
            # waits on the ACT sqrt.
            psq = pma = 0
            for t in range(NT):
                scan_tick(t)
                if psq + TAILB <= t:
                    tail_sqrt(psq, TAILB)
                    psq += TAILB
                if pma + TAILB <= t - TAILLAG:
                    tail_ma(pma, TAILB)
                    dma_out(pma, TAILB)
                    pma += TAILB
            while psq < NT:
                n = min(TAILB, NT - psq)
                tail_sqrt(psq, n)
                psq += n
            while pma < NT:
                n = min(TAILB, NT - pma)
                tail_ma(pma, n)
                dma_out(pma, n)
                pma += n
    if not nc.is_finalized():
        nc.finalize()
    return nc


# ---------------------------------------------------------------- tracing
def _maybe_enable_trace():
    if os.environ.get("BASS_TRACE") != "1":
        return
    try:
        import sys, types
        try:
            import antenv.axon_hooks as ah
        except ImportError:
            import antenv
            ah = types.ModuleType("antenv.axon_hooks")
            ah._hook = None
            def _set(h):
                ah._hook = h
            def _get():
                return ah._hook
            ah.set_axon_ntff_profile_hook = _set
            ah.get_axon_ntff_profile_hook = _get
            sys.modules["antenv.axon_hooks"] = ah
            antenv.axon_hooks = ah
        if ah.get_axon_ntff_profile_hook() is not None:
            return
        from trn_agent_boot.trn_boot import _ntff_profile_via_ctypes
        import concourse.bass_utils as bu
        bu.upload_artifacts = lambda tmpdir: tmpdir
        ah.set_axon_ntff_profile_hook(
            _ntff_profile_via_ctypes("/opt/axon/libaxon_pjrt.so"))
        print("[kernel] NTFF profile hook installed")
    except Exception as e:
        print(f"[kernel] trace hook unavailable: {e}")


LAST = None  # last BassKernelResults (dev/tracing)


# ---------------------------------------------------------------- entry
def kernel(**inputs):
    import ml_dtypes
    bfl = ml_dtypes.bfloat16
    _maybe_enable_trace()
    x = np.asarray(inputs["x"], np.float32)
    last_mu = np.asarray(inputs["last_mu"], np.float32)
    last_sigma = np.asarray(inputs["last_sigma"], np.float32)
    sc = dict(
        a_mu=float(inputs["alpha_mu"]), a_s=float(inputs["alpha_sigma"]),
        b_mu=float(inputs["beta_mu"]), b_s=float(inputs["beta_sigma"]),
        o_mu=float(inputs["omega_mu"]), o_s=float(inputs["omega_sigma"]),
        nu=float(inputs["nu"]), ns=float(inputs["norm_strength"]),
    )
    # biases are structurally zero in this net (setup_inputs); the padded
    # no-bias-row layout depends on it.
    for bn in ("b1", "b2", "b3"):
        assert float(np.abs(np.asarray(inputs[bn])).max()) == 0.0, \
            f"{bn} != 0 unsupported by padded kernel"

    def pad(w, rows, cols):
        out = np.zeros((rows, cols), np.float32)
        a = np.asarray(w, np.float32)
        out[:a.shape[0], :a.shape[1]] = a
        return out.astype(bfl)

    W1e = pad(inputs["W1"], D_IN, HP)
    W2e = pad(inputs["W2"], HP, HP)
    W3e = pad(inputs["W3"], HP, K)

    nc = build_nc(sc)
    in_maps = []
    for cidx in range(NCORES):
        sl = slice(cidx * BC, (cidx + 1) * BC)
        # lane inits: [P, LW] each, col = row-tile within lane
        lm = last_mu[sl].reshape(2, LW, P)          # [lane, tile, p]
        ls = (sc["nu"] * last_sigma[sl]).reshape(2, LW, P)
        mu0 = np.concatenate([lm[0].T, lm[1].T], axis=1)   # [P, 2*LW]
        s20 = np.concatenate([ls[0].T, ls[1].T], axis=1)
        in_maps.append({
            "xT": np.ascontiguousarray(x[sl].T).astype(bfl),
            "W1e": W1e, "W2e": W2e, "W3e": W3e,
            "mu0": np.ascontiguousarray(mu0).astype(bfl),
            "s20": np.ascontiguousarray(s20).astype(bfl),
        })
    res = run_bass_kernel_spmd(nc, in_maps, list(range(NCORES)))
    global LAST
    LAST = res
    if res.exec_time_ns is not None:
        print(f"HW exec time: {res.exec_time_ns} ns")
    # out[p, t*256 + half*LW + c] -> full[(half*LW+c)*P + p, k]
    parts = []
    for i in range(NCORES):
        o = np.asarray(res.results[i]["out"]).astype(np.float32)
        o = o.reshape(P, NT, 2, LW)
        l0 = o[:, 0:K, 0, :].transpose(2, 0, 1).reshape(LW * P, K)
        l1 = o[:, S:S + K, 1, :].transpose(2, 0, 1).reshape(LW * P, K)
        parts.append(l0)
        parts.append(l1)
    return np.concatenate(parts, 0)
